# Initial kernel scaffold
#
"""Trainium2 8-core Bass kernel for nn_AI4Urban (CFD step + multigrid).

Self-contained: builds per-call (weights/dt baked as compile-time consts),
shards the 128^3 grid along z across 8 NeuronCores with 3-deep ghost input
planes, runs all 3x3x3 stencils as banded f32r matmuls on the PE
(x in partitions, (z,y) in the free dim), does the multigrid coarse levels
replicated below 64^3 with one AllGather at the 32^3 level plus one
indirect-DMA z-slice per iteration, and exchanges a 1-plane p halo per MG
iteration via AllGather + per-core index gather.
"""
import sys
sys.path.insert(0, '/opt/trn_rl_repo')
import numpy as np

from concourse import bacc, bass, tile, bass_utils, mybir

NC = 8
D = 128
ZL = D // NC        # 16 local planes
G = 3               # ghost depth of input tiles
ZX = ZL + 2 * G     # 22-slot global frame
YP = 130

f32 = mybir.dt.float32
f32r = mybir.dt.float32r
i32 = mybir.dt.int32
AF = mybir.ActivationFunctionType
ALU = mybir.AluOpType

# mats128 layout: name -> (base index, count)
_M128 = {}
_n = 0
for _nm in ('xp', 'yp', 'zp', 'dp_c2', 'dp_c2b', 'Ap',
            'xm', 'ym', 'zm', 'dm_c2', 'dm_c2b'):
    _M128[_nm] = (_n, 9)
    _n += 9
for _nm in ('I1', 'Ic1', 'Ic1b', 'Imdiag', 'Ipdiag', 'Iminvdt'):
    _M128[_nm] = (_n, 1)
    _n += 1
N_M128 = _n
MINUS_STENCILS = ('x', 'y', 'z', 'd')   # order for L12 / LB packing


# ------------------------------------------------------------------ host math
def _band(w, f, size=D, fold=True):
    B = (w[0] * np.eye(size, k=1) + w[1] * np.eye(size) + w[2] * np.eye(size, k=-1))
    if fold:
        B[0, 0] += f * w[0]
        B[size - 1, size - 1] += f * w[2]
    return B.astype(np.float32)


def _band_set(w3, f, scale=1.0, size=D, fold=True):
    out = np.zeros((9, size, size), np.float32)
    for dz in range(3):
        for dy in range(3):
            out[dz * 3 + dy] = scale * _band(w3[dz, dy], f, size, fold)
    return out


def _res_set(w_res, s_in):
    so = s_in // 2
    out = np.zeros((4, s_in, so), np.float32)
    for dz in range(2):
        for dy in range(2):
            for m in range(so):
                for dx in range(2):
                    out[dz * 2 + dy, 2 * m + dx, m] = w_res[dz, dy, dx]
    return out


def _prol_mat(s):
    P = np.zeros((s, 2 * s), np.float32)
    for k in range(s):
        P[k, 2 * k] = 1.0
        P[k, 2 * k + 1] = 1.0
    return P


def _host_prep(inputs):
    """Build per-core in_maps + compile-key constants."""
    gv = lambda k: np.asarray(inputs[k], np.float32).reshape(
        inputs[k].shape[-3:] if np.ndim(inputs[k]) >= 3 else np.shape(inputs[k]))
    vu, vv_, vw, vp = gv('values_u'), gv('values_v'), gv('values_w'), gv('values_p')
    sg = gv('sigma')
    w_x = np.asarray(inputs['w_xadv'], np.float64).reshape(3, 3, 3)
    w_y = np.asarray(inputs['w_yadv'], np.float64).reshape(3, 3, 3)
    w_z = np.asarray(inputs['w_zadv'], np.float64).reshape(3, 3, 3)
    w_d = np.asarray(inputs['w_diff'], np.float64).reshape(3, 3, 3)
    wA = np.asarray(inputs['wA'], np.float64).reshape(3, 3, 3)
    w_res = np.asarray(inputs['w_res'], np.float64).reshape(2, 2, 2)
    dt = float(np.asarray(inputs['dt']).reshape(-1)[0])
    iteration = int(inputs['iteration'])
    nlevel = int(inputs['nlevel'])

    S = float(w_d.sum())
    diag = float(wA[1, 1, 1])
    c1 = 1.0 - 0.00025 * dt * S
    c2 = 0.0005 * dt
    c2b = 0.001 * dt
    c1b = -0.0005 * dt * S

    consts = dict(dt=dt, S=S, diag=diag, c1=c1, c2=c2, c2b=c2b, c1b=c1b,
                  iteration=iteration, nlevel=nlevel)

    # ---- shared matrices ----
    m128 = np.zeros((N_M128, D, D), np.float32)
    m128[_M128['xp'][0]:_M128['xp'][0] + 9] = _band_set(w_x, 1.0)
    m128[_M128['yp'][0]:_M128['yp'][0] + 9] = _band_set(w_y, 1.0)
    m128[_M128['zp'][0]:_M128['zp'][0] + 9] = _band_set(w_z, 1.0)
    m128[_M128['dp_c2'][0]:_M128['dp_c2'][0] + 9] = _band_set(w_d, 1.0, c2)
    m128[_M128['dp_c2b'][0]:_M128['dp_c2b'][0] + 9] = _band_set(w_d, 1.0, c2b)
    m128[_M128['Ap'][0]:_M128['Ap'][0] + 9] = _band_set(wA, 1.0)
    m128[_M128['xm'][0]:_M128['xm'][0] + 9] = _band_set(w_x, -1.0)
    m128[_M128['ym'][0]:_M128['ym'][0] + 9] = _band_set(w_y, -1.0)
    m128[_M128['zm'][0]:_M128['zm'][0] + 9] = _band_set(w_z, -1.0)
    m128[_M128['dm_c2'][0]:_M128['dm_c2'][0] + 9] = _band_set(w_d, -1.0, c2)
    m128[_M128['dm_c2b'][0]:_M128['dm_c2b'][0] + 9] = _band_set(w_d, -1.0, c2b)
    I = np.eye(D, dtype=np.float32)
    m128[_M128['I1'][0]] = I
    m128[_M128['Ic1'][0]] = c1 * I
    m128[_M128['Ic1b'][0]] = c1b * I
    m128[_M128['Imdiag'][0]] = (-1.0 / diag) * I
    m128[_M128['Ipdiag'][0]] = (1.0 / diag) * I
    m128[_M128['Iminvdt'][0]] = (-1.0 / dt) * I

    res0 = _res_set(w_res, 128)
    res1 = _res_set(w_res, 64)
    resc = {s: _res_set(w_res, s) for s in (32, 16, 8, 4, 2)}
    # coarse A sets: [11, s, s]: 9 x A*(-1/diag) zero-fold + I + I/diag
    mco = {}
    for s in (64, 32, 16, 8, 4, 2):
        m = np.zeros((11, s, s), np.float32)
        m[:9] = _band_set(wA, 0.0, -1.0 / diag, s, fold=False)
        m[9] = np.eye(s, dtype=np.float32)
        m[10] = np.eye(s, dtype=np.float32) / diag
        mco[s] = m
    prols = {s: _prol_mat(s) for s in (1, 2, 4, 8, 16, 32)}
    prol64p = _prol_mat(64)
    prol64n = -prol64p

    # corr (a) matrices: L12[s, yc] [12, 128]; row k = dz*4 + yc_r*2 + xe
    ws = {'x': w_x, 'y': w_y, 'z': w_z, 'd': w_d}
    L12 = np.zeros((4, 2, 12, D), np.float32)
    for si, sn in enumerate(MINUS_STENCILS):
        w3 = ws[sn]
        for yc in range(2):
            dyt = 0 if yc == 0 else 2
            for dz in range(3):
                for xe in range(2):
                    m = 0 if xe == 0 else D - 1
                    xet = 0 if xe == 0 else 2
                    L12[si, yc, dz * 4 + yc * 2 + xe, m] = 2.0 * w3[dz, dyt, xet]

    base = dict(m128=m128, res0=res0, res1=res1, prol64p=prol64p, prol64n=prol64n)
    for s, m in resc.items():
        base[f'resc{s}'] = m
    for s, m in mco.items():
        base[f'mco{s}'] = m
    for s, m in prols.items():
        base[f'prol{s}'] = m
    base['L12'] = L12

    # ---- per-core data ----
    def shard(gf):
        out = []
        for c in range(NC):
            zmin = c * ZL - G
            idx = np.clip(np.arange(zmin, zmin + ZX), 0, D - 1)
            out.append(np.ascontiguousarray(
                np.transpose(gf[idx], (2, 0, 1)).astype(np.float32)))  # [x, z, y]
        return out

    sh_u, sh_v, sh_w, sh_p, sh_s = (shard(a) for a in (vu, vv_, vw, vp, sg))

    in_maps = []
    for c in range(NC):
        is_bot = c == 0
        is_top = c == NC - 1
        vm = np.zeros(10, np.float32)
        vm[0] = 0.0 if is_bot else 1.0           # m_bot
        vm[1] = -1.0 if is_bot else 0.0          # s_bot minus
        vm[2] = 1.0 if is_bot else 0.0           # s_bot plus
        vm[3] = 0.0 if is_top else 1.0
        vm[4] = -1.0 if is_top else 0.0
        vm[5] = 1.0 if is_top else 0.0
        vm[6] = -1.0 if is_bot else 1.0          # zsgn bot (u)
        vm[7] = -1.0 if is_top else 1.0
        vm[8] = 1.0 if is_bot else -1.0          # ycol ghost scale minus, bot
        vm[9] = 1.0 if is_top else -1.0

        # corr (b) matrices LB[s, side, dy] [2, 128], mask-baked
        LB = np.zeros((4, 2, 3, 2, D), np.float32)
        for si, sn in enumerate(MINUS_STENCILS):
            w3 = ws[sn]
            for side in range(2):
                mask = 1.0 if (is_bot if side == 0 else is_top) else 0.0
                dzt = 0 if side == 0 else 2
                for dy in range(3):
                    LB[si, side, dy, 0, 0] = mask * 2.0 * w3[dzt, dy, 0]
                    LB[si, side, dy, 1, D - 1] = mask * 2.0 * w3[dzt, dy, 2]

        # p-halo gather rows into view [(16*128), 128]: plane row = pl*128 + x
        xs = np.arange(D, dtype=np.int32)
        bot_pl = 2 * (c - 1) + 1 if c > 0 else 0       # top plane of c-1, else own bottom
        top_pl = 2 * (c + 1) + 0 if c < NC - 1 else 2 * c + 1
        idx_ph = np.stack([bot_pl * D + xs, top_pl * D + xs], 1).astype(np.int32)

        # wmg32 slice: planes 4c-1 .. 4c+4 of [32,32,32] in view [(32*32), 32]
        idx_w = np.zeros((32, 6), np.int32)
        for j, z in enumerate(range(4 * c - 1, 4 * c + 5)):
            idx_w[:, j] = (z * 32 + np.arange(32)) if 0 <= z < 32 else 10 ** 6

        m = dict(base)
        m.update(fld_u=sh_u[c], fld_v=sh_v[c], fld_w=sh_w[c], fld_p=sh_p[c],
                 fld_s=sh_s[c], vm=vm, LB=LB, idx_ph=idx_ph, idx_w=idx_w)
        in_maps.append(m)

    return in_maps, consts


# ------------------------------------------------------------------ builder
def _build(consts):
    dt = consts['dt']
    diag = consts['diag']
    iteration = consts['iteration']

    nc = bacc.Bacc("TRN2", target_bir_lowering=False, debug=False, num_devices=NC)

    dr = {}
    def din(name, shape, dtp=f32r):
        dr[name] = nc.dram_tensor(name, list(shape), dtp, kind="ExternalInput")
        return dr[name]

    for nm in ('fld_u', 'fld_v', 'fld_w', 'fld_p'):
        din(nm, (D, ZX, D))
    din('fld_s', (D, ZX, D), f32)
    din('m128', (N_M128, D, D))
    din('res0', (4, 128, 64)); din('res1', (4, 64, 32))
    for s in (32, 16, 8, 4, 2):
        din(f'resc{s}', (4, s, s // 2))
    for s in (64, 32, 16, 8, 4, 2):
        din(f'mco{s}', (11, s, s))
    for s in (1, 2, 4, 8, 16, 32):
        din(f'prol{s}', (s, 2 * s))
    din('prol64p', (64, 128)); din('prol64n', (64, 128))
    din('L12', (4, 2, 12, D)); din('LB', (4, 2, 3, 2, D))
    din('vm', (10,), f32)
    din('idx_ph', (D, 2), i32); din('idx_w', (32, 6), i32)

    out_f = nc.dram_tensor("out_fields", [5, D, ZL, D], f32, kind="ExternalOutput")
    out_r = nc.dram_tensor("out_r", [1, 1], f32, kind="ExternalOutput")

    DMAE = [nc.sync, nc.scalar, nc.vector, nc.gpsimd, nc.tensor]
    dma_i = [0]
    def dma(dst, src):
        e = DMAE[dma_i[0] % len(DMAE)]
        dma_i[0] += 1
        e.dma_start(dst, src)

    with tile.TileContext(nc) as tc:
        with (
            tc.tile_pool(name="pool", bufs=1) as P_,
            tc.tile_pool(name="mats", bufs=4) as MP,
            tc.tile_pool(name="tmp", bufs=2) as TP,
            tc.tile_pool(name="psum", bufs=8, space="PSUM") as PS,
            tc.tile_pool(name="psc", bufs=4, space="PSUM") as PSC,
            tc.tile_pool(name="dram", bufs=1, space="DRAM") as DP,
        ):
            r32 = lambda ap: ap.bitcast(f32)

            # ---------- resident small tiles ----------
            ids = P_.tile([D, 6, D], f32r, tag="ids")
            dma(ids[:], dr['m128'][_M128['I1'][0]:_M128['I1'][0] + 6]
                .rearrange("i k m -> k i m"))
            def ID(nm):
                return ids[:, _M128[nm][0] - _M128['I1'][0], :]

            rs0 = P_.tile([128, 4, 64], f32r, tag="rs0")
            dma(rs0[:], dr['res0'][:].rearrange("i k m -> k i m"))
            rs1 = P_.tile([64, 4, 32], f32r, tag="rs1")
            dma(rs1[:], dr['res1'][:].rearrange("i k m -> k i m"))
            rsc = {}
            for s in (32, 16, 8, 4, 2):
                rsc[s] = P_.tile([s, 4, s // 2], f32r, tag=f"rsc{s}")
                dma(rsc[s][:], dr[f'resc{s}'][:].rearrange("i k m -> k i m"))
            mco = {}
            for s in (64, 32, 16, 8, 4, 2):
                mco[s] = P_.tile([s, 11, s], f32r, tag=f"mco{s}")
                dma(mco[s][:], dr[f'mco{s}'][:].rearrange("i k m -> k i m"))
            prl = {}
            for s in (1, 2, 4, 8, 16, 32):
                prl[s] = P_.tile([s, 2 * s], f32r, tag=f"prl{s}")
                dma(prl[s][:], dr[f'prol{s}'][:])
            p64p = P_.tile([64, 128], f32r, tag="p64p")
            dma(p64p[:], dr['prol64p'][:])
            p64n = P_.tile([64, 128], f32r, tag="p64n")
            dma(p64n[:], dr['prol64n'][:])
            L12 = P_.tile([12, 8, D], f32r, tag="L12")
            dma(L12[:], dr['L12'][:].rearrange("s c k m -> k (s c) m"))
            LB = P_.tile([2, 24, D], f32r, tag="LB")
            dma(LB[:], dr['LB'][:].rearrange("s e y k m -> k (s e y) m"))
            vm = P_.tile([D, 10], f32, tag="vm")
            dma(vm[:], dr['vm'][:].to_broadcast([D, 10]))
            ixp = P_.tile([D, 2], i32, tag="ixp")
            dma(ixp[:], dr['idx_ph'][:])
            ixw = P_.tile([32, 6], i32, tag="ixw")
            dma(ixw[:], dr['idx_w'][:])
            apm = MP.tile([D, 9, D], f32r, tag="apm")   # A-plus set, resident
            dma(apm[:], dr['m128'][_M128['Ap'][0]:_M128['Ap'][0] + 9]
                .rearrange("i k m -> k i m"))

            # ---------- field tiles ----------
            def ftile(nm, z0, zn, tag=None):
                t = P_.tile([D, zn, YP], f32r, tag=tag or nm)
                t.z0 = z0
                return t

            U = {n: ftile('t' + n, 0, ZX) for n in 'uvw'}
            p = ftile('p', 0, ZX)
            rd = P_.tile([D, ZX, D], f32, tag="rd")
            B_ = {n: ftile('b' + n, 1, 20) for n in 'uvw'}
            U2 = {n: ftile('q' + n, 2, 18) for n in 'uvw'}
            xps = {n: P_.tile([D, 20, D], f32, tag='xp' + n) for n in 'uvw'}

            # load raw fields into [:, :, 1:129]
            for n, src in (('u', 'fld_u'), ('v', 'fld_v'), ('w', 'fld_w')):
                dma(U[n][:, :, 1:129], dr[src][:])
            dma(p[:, :, 1:129], dr['fld_p'][:])
            sgt = TP.tile([D, ZX, D], f32, tag="sg")
            dma(sgt[:], dr['fld_s'][:])

            # rd = 1/(1+dt*sigma)
            nc.vector.tensor_scalar(rd[:], sgt[:], float(dt), 1.0,
                                    op0=ALU.mult, op1=ALU.add)
            nc.vector.reciprocal(rd[:], rd[:])

            # u = values_u * rd (in place)
            for n in 'uvw':
                nc.vector.tensor_mul(U[n][:, :, 1:129], r32(U[n][:, :, 1:129]), rd[:])

            # z-ghost sign flip for u (per-core scale)
            for (sl, vi) in ((slice(0, G), 6), (slice(ZX - G, ZX), 7)):
                nc.scalar.activation(U['u'][:, sl, 1:129], r32(U['u'][:, sl, 1:129]),
                                     AF.Copy, scale=vm[:, vi:vi + 1])

            def ycols(t, z0, a, b, f, gslots=()):
                """fill y halo cols for global slots [a,b) with factor f (imm);
                gslots: list of (slot, vm_idx) with per-core scale."""
                la, lb = a - z0, b - z0
                nc.scalar.activation(t[:, la:lb, 0:1], r32(t[:, la:lb, 1:2]),
                                     AF.Copy, scale=float(f))
                nc.scalar.activation(t[:, la:lb, 129:130], r32(t[:, la:lb, 128:129]),
                                     AF.Copy, scale=float(f))
                for (g, vi) in gslots:
                    lg = g - z0
                    nc.scalar.activation(t[:, lg:lg + 1, 0:1], r32(t[:, lg:lg + 1, 1:2]),
                                         AF.Copy, scale=vm[:, vi:vi + 1])
                    nc.scalar.activation(t[:, lg:lg + 1, 129:130],
                                         r32(t[:, lg:lg + 1, 128:129]),
                                         AF.Copy, scale=vm[:, vi:vi + 1])

            ycols(U['u'], 0, G, ZX - G, -1.0,
                  gslots=[(g, 8) for g in range(G)] + [(g, 9) for g in range(ZX - G, ZX)])
            ycols(U['v'], 0, 0, ZX, 1.0)
            ycols(U['w'], 0, 0, ZX, 1.0)
            ycols(p, 0, 0, ZX, 1.0)

            # CC3 tiles for minus fields: [12, Zout]; row k=dz*4+yc*2+xe
            def cc3(t, z0, o0, zout):
                c = TP.tile([12, zout], f32r, tag="cc3")
                for dz in range(3):
                    for yc in range(2):
                        for xe in range(2):
                            k = dz * 4 + yc * 2 + xe
                            m = 0 if xe == 0 else D - 1
                            ycol = 0 if yc == 0 else 129
                            ls = o0 - 1 + dz - z0
                            nc.scalar.activation(
                                c[k:k + 1, :],
                                r32(t[m:m + 1, ls:ls + zout, ycol:ycol + 1]),
                                AF.Copy)
                return c

            def mset(nm):
                t = MP.tile([D, 9, D], f32r, tag="mset")
                b0 = _M128[nm][0]
                dma(t[:], dr['m128'][b0:b0 + 9].rearrange("i k m -> k i m"))
                return t

            # ---------------- conv machinery ----------------
            def conv_groups(T, o0, zout, mt, extra=None, corr=None, psum_pool=PS):
                """Yield (chunk_index, o_ck(global), cp, psum_tile, finish_fn).
                Caller must consume psum then let it free.  mt: [128,9,128] set.
                extra: list of (lhsT_ap, rhs_fn(a, cp)) appended to the group.
                corr: (si, cc3_tile, o0_cc, src_T) to add corner fixes."""
                for ci, a in enumerate(range(0, zout, 4)):
                    cp = min(4, zout - a)
                    ps = psum_pool.tile([D, 4, D], f32, tag="cv")
                    ops = []
                    for dz in range(3):
                        for dy in range(3):
                            ls = o0 + a - 1 + dz - T.z0
                            ops.append((mt[:, dz * 3 + dy, :],
                                        T[:, ls:ls + cp, dy:dy + D]))
                    if extra:
                        for (lh, rhfn) in extra:
                            ops.append((lh, rhfn(a, cp)))
                    if corr is not None:
                        si, cct, cco0, srcT = corr
                        for yc in range(2):
                            yo = 0 if yc == 0 else D - 1
                            ops.append((L12[:, si * 2 + yc, :],
                                        cct[:, a:a + cp],
                                        ps[:, 0:cp, yo:yo + 1]))
                        # side fixes at first/last interior plane
                        for side, og in ((0, G), (1, G + ZL - 1)):
                            pos = og - o0 - a
                            if not (0 <= pos < cp):
                                continue
                            gs = (G - 1 if side == 0 else ZX - G) - srcT.z0
                            for dy in range(3):
                                col = si * 6 + side * 3 + dy
                                ops.append((LB[:, col, :],
                                            srcT[:, gs, dy:dy + D].unsqueeze(1)
                                            if False else
                                            srcT[:, gs:gs + 1, dy:dy + D],
                                            ps[:, pos:pos + 1, :]))
                    n = len(ops)
                    for i, op in enumerate(ops):
                        if len(op) == 2:
                            lh, rh = op
                            o_ap = ps[:, 0:cp, :]
                        else:
                            lh, rh, o_ap = op
                        nc.tensor.matmul(o_ap, lh, rh,
                                         start=(i == 0), stop=(i == n - 1))
                    yield ci, o0 + a, cp, ps

            def full_conv(T, o0, zout, mt, dst, scale=1.0, extra=None, corr=None):
                """conv -> evict into dst[:, (o-zoff), 1:129 or 0:128]."""
                for ci, og, cp, ps in conv_groups(T, o0, zout, mt, extra, corr):
                    lo = og - dst.z0 if hasattr(dst, 'z0') else og - o0
                    d = dst[:, lo:lo + cp, 1:129] if dst.shape[2] == YP \
                        else dst[:, lo:lo + cp, :]
                    nc.scalar.activation(d, ps[:, 0:cp, :], AF.Copy, scale=float(scale))

            # ---------------- stage 1 ----------------
            # pp advection convs (plus, raw; evict scale=-dt)
            for sn, n in (('xp', 'u'), ('yp', 'v'), ('zp', 'w')):
                mt = mset(sn)
                xx = xps[n]
                xx.z0 = 1
                full_conv(p, 1, 20, mt, xx, scale=-dt)

            mtx = {sn: mset(sn) for sn in ('xm', 'ym', 'zm')}
            mtd = mset('dm_c2')
            mtxp = {sn: mset(sn) for sn in ('xp', 'yp', 'zp')}
            mtdp = mset('dp_c2')

            cc_u = cc3(U['u'], 0, 1, 20)

            def stage_combine(comp, T, o0, zout, Usrc, msets, mdf, Iid, Ibase2,
                              dtc, xpsrc, dst, cc, stage2=False):
                """comp in 'uvw'; builds dst = (base + c*diff - dtc*madv - dt*xp)*rd."""
                is_minus = comp == 'u'
                mtile = TP.tile([D, zout, D], f32, tag="madv")
                # advection products
                for k, (sn, mul) in enumerate((('x', 'u'), ('y', 'v'), ('z', 'w'))):
                    mt = msets[sn]
                    corr = ( MINUS_STENCILS.index(sn), cc, o0, T) if is_minus else None
                    for ci, og, cp, ps in conv_groups(T, o0, zout, mt, corr=corr):
                        lo = og - o0
                        mu = Usrc[mul]
                        msl = mu[:, og - mu.z0: og - mu.z0 + cp, 1:129]
                        if k == 0:
                            nc.vector.tensor_mul(mtile[:, lo:lo + cp, :],
                                                 r32(msl), ps[:, 0:cp, :])
                        else:
                            tt = TP.tile([D, 4, D], f32, tag="advt")
                            nc.vector.tensor_mul(tt[:, 0:cp, :], r32(msl),
                                                 ps[:, 0:cp, :])
                            nc.gpsimd.tensor_add(mtile[:, lo:lo + cp, :],
                                                 mtile[:, lo:lo + cp, :],
                                                 tt[:, 0:cp, :])
                # diff + identity group -> dst pre
                ex = [(Iid, lambda a, cp, _m=Usrc[comp]:
                       _m[:, o0 + a - _m.z0: o0 + a - _m.z0 + cp, 1:129])]
                if Ibase2 is not None:
                    ex.append((Ibase2, lambda a, cp, _m=T:
                               _m[:, o0 + a - _m.z0: o0 + a - _m.z0 + cp, 1:129]))
                corr = (3, cc, o0, T) if is_minus else None
                for ci, og, cp, ps in conv_groups(T, o0, zout, mdf, extra=ex, corr=corr):
                    lo = og - o0
                    # pre = (madv * -dtc) + psum
                    nc.vector.scalar_tensor_tensor(
                        mtile[:, lo:lo + cp, :], mtile[:, lo:lo + cp, :],
                        float(-dtc), ps[:, 0:cp, :], op0=ALU.mult, op1=ALU.add)
                # + xp_s (already -dt scaled), then * rd
                nc.gpsimd.tensor_add(mtile[:], mtile[:],
                                     xpsrc[:, o0 - xpsrc.z0:o0 - xpsrc.z0 + zout, :])
                nc.vector.tensor_mul(dst[:, o0 - dst.z0:o0 - dst.z0 + zout, 1:129],
                                     mtile[:], rd[:, o0:o0 + zout, :])

            for comp in 'uvw':
                T = U[comp]
                msets = {'x': mtx['xm'], 'y': mtx['ym'], 'z': mtx['zm']} \
                    if comp == 'u' else {'x': mtxp['xp'], 'y': mtxp['yp'], 'z': mtxp['zp']}
                mdf = mtd if comp == 'u' else mtdp
                stage_combine(comp, T, 1, 20, U, msets, mdf, ID('Ic1'), None,
                              0.5 * dt, xps[comp], B_[comp], cc_u if comp == 'u' else None)

            # blend boundary ghosts of b_*
            def blend(t, slots_bot, slots_top, minus):
                eb = t[:, G - t.z0, 1:129]
                et = t[:, G + ZL - 1 - t.z0, 1:129]
                for (slots, edge, mvi, svi) in (
                        (slots_bot, eb, 0, 1 if minus else 2),
                        (slots_top, et, 3, 4 if minus else 5)):
                    for g in slots:
                        sl = t[:, g - t.z0, 1:129]
                        nc.scalar.activation(sl, r32(sl), AF.Copy,
                                             scale=vm[:, mvi:mvi + 1])
                        nc.vector.scalar_tensor_tensor(
                            sl, r32(edge), vm[:, svi:svi + 1], r32(sl),
                            op0=ALU.mult, op1=ALU.add)

            blend(B_['u'], (1, 2), (19, 20), True)
            blend(B_['v'], (1, 2), (19, 20), False)
            blend(B_['w'], (1, 2), (19, 20), False)
            ycols(B_['u'], 1, G, ZX - G, -1.0,
                  gslots=[(1, 8), (2, 8), (19, 9), (20, 9)])
            ycols(B_['v'], 1, 1, 21, 1.0)
            ycols(B_['w'], 1, 1, 21, 1.0)

            # ---------------- stage 2 ----------------
            mtx2 = {sn: mset(sn) for sn in ('xm', 'ym', 'zm')}
            mtd2 = mset('dm_c2b')
            mtxp2 = {sn: mset(sn) for sn in ('xp', 'yp', 'zp')}
            mtdp2 = mset('dp_c2b')
            cc_b = cc3(B_['u'], 1, 2, 18)
            for comp in 'uvw':
                T = B_[comp]
                msets = {'x': mtx2['xm'], 'y': mtx2['ym'], 'z': mtx2['zm']} \
                    if comp == 'u' else {'x': mtxp2['xp'], 'y': mtxp2['yp'], 'z': mtxp2['zp']}
                mdf = mtd2 if comp == 'u' else mtdp2
                xpv = xps[comp]
                stage_combine(comp, T, 2, 18, B_, msets, mdf, ID('Ic1b'), ID('I1'),
                              dt, xpv, U2[comp], cc_b if comp == 'u' else None)

            # note: stage-2 Usrc for identity base is U (u), advection mults are b_*
            # -> handled inside stage_combine via Usrc[comp] (B_) ... fixed below.

            blend(U2['u'], (2,), (19,), True)
            blend(U2['v'], (2,), (19,), False)
            blend(U2['w'], (2,), (19,), False)
            ycols(U2['u'], 2, G, ZX - G, -1.0, gslots=[(2, 8), (19, 9)])
            ycols(U2['v'], 2, 2, 20, 1.0)
            ycols(U2['w'], 2, 2, 20, 1.0)

            # ---------------- stage 3: b ----------------
            b = P_.tile([D, ZL, D], f32r, tag="bu")   # reuse-size ok
            b.z0 = G
            mt_x3 = mset('xm')
            mt_y3 = mset('yp')
            mt_z3 = mset('zp')
            cc_q = cc3(U2['u'], 2, 3, 16)
            for ci, a in enumerate(range(0, ZL, 4)):
                cp = 4
                og = G + a
                ps = PS.tile([D, 4, D], f32, tag="cv")
                ops = []
                for (mt, T) in ((mt_x3, U2['u']), (mt_y3, U2['v']), (mt_z3, U2['w'])):
                    for dz in range(3):
                        for dy in range(3):
                            ls = og - 1 + dz - T.z0
                            ops.append((mt[:, dz * 3 + dy, :],
                                        T[:, ls:ls + cp, dy:dy + D], ps[:, 0:cp, :]))
                # corr (a) for xm on u2
                for yc in range(2):
                    yo = 0 if yc == 0 else D - 1
                    ops.append((L12[:, 0 * 2 + yc, :], cc_q[:, a:a + cp],
                                ps[:, 0:cp, yo:yo + 1]))
                for side, ogx in ((0, G), (1, G + ZL - 1)):
                    pos = ogx - og
                    if 0 <= pos < cp:
                        gs = (G - 1 if side == 0 else ZX - G) - U2['u'].z0
                        for dy in range(3):
                            ops.append((LB[:, 0 * 6 + side * 3 + dy, :],
                                        U2['u'][:, gs:gs + 1, dy:dy + D],
                                        ps[:, pos:pos + 1, :]))
                n = len(ops)
                for i, (lh, rh, o_ap) in enumerate(ops):
                    nc.tensor.matmul(o_ap, lh, rh, start=(i == 0), stop=(i == n - 1))
                nc.scalar.activation(b[:, a:a + cp, :], ps[:, 0:cp, :], AF.Copy,
                                     scale=float(-1.0 / dt))

            # ---------------- multigrid ----------------
            App = P_.tile([D, ZL, D], f32r, tag="bv")
            App.z0 = G
            wmg = P_.tile([D, ZL, D], f32, tag="bw")
            wmg.z0 = G

            # DRAM bounces
            ph_in = DP.tile([2 * D * D], f32, tag="phin")
            ph_out = DP.tile([NC, 2 * D * D], f32, tag="phout", addr_space="Shared")
            r2_in = DP.tile([4 * 32 * 32], f32, tag="r2in")
            r2_out = DP.tile([NC, 4 * 32 * 32], f32, tag="r2out", addr_space="Shared")
            w32_d = DP.tile([32 * 32, 32], f32, tag="w32d")

            # coarse pads (zero borders persist)
            wpad = {}
            for s in (2, 4, 8, 16, 32):
                wpad[s] = P_.tile([s, s + 2, s + 2], f32r, tag=f"wp{s}")
                nc.vector.memset(wpad[s][:], 0.0)
            w64pad = P_.tile([64, 12, 66], f32r, tag="w64p")
            nc.vector.memset(w64pad[:], 0.0)
            w32sl = P_.tile([32, 6, 32], f32r, tag="w32sl")

            r7_out = P_.tile([1, 1], f32, tag="r7o")

            for it in range(iteration):
                # A(pp) -> App
                full_conv(p, G, ZL, apm, App)
                # r0 = App - b
                rzero = TP.tile([D, ZL, D], f32r, tag="rz")
                nc.vector.tensor_sub(rzero[:], r32(App[:, :, :]).bitcast(f32),
                                     r32(b[:, :, :]).bitcast(f32))
                # restrict r0 -> r1 [64, 8, 64]
                r1 = TP.tile([64, 8, 64], f32r, tag="r1")
                ps = PSC.tile([64, 8, 64], f32, tag="c64")
                for i in range(4):
                    dz, dy = i // 2, i % 2
                    nc.tensor.matmul(ps[:], rs0[:, i, :],
                                     rzero[:, dz::2, dy::2][:, 0:8, 0:64],
                                     start=(i == 0), stop=(i == 3))
                nc.scalar.activation(r1[:], ps[:], AF.Copy)
                # restrict r1 -> r2 [32, 4, 32]
                r2 = TP.tile([32, 4, 32], f32r, tag="r2")
                ps = PSC.tile([32, 4, 32], f32, tag="c32")
                for i in range(4):
                    dz, dy = i // 2, i % 2
                    nc.tensor.matmul(ps[:], rs1[:, i, :],
                                     r1[:, dz::2, dy::2][:, 0:4, 0:32],
                                     start=(i == 0), stop=(i == 3))
                nc.scalar.activation(r2[:], ps[:], AF.Copy)
                # AllGather r2
                nc.sync.dma_start(
                    r2_in[:].rearrange("(z x y) -> x z y", z=4, x=32, y=32),
                    r32(r2[:]))
                nc.gpsimd.collective_compute(
                    "AllGather", ALU.bypass, replica_groups=[list(range(NC))],
                    ins=[r2_in[:].opt()], outs=[r2_out[:].opt()])
                r2f = TP.tile([32, 32, 32], f32r, tag="r2f")
                nc.sync.dma_start(
                    r2f[:],
                    r2_out[:].rearrange("c (z x y) -> x (c z) y", z=4, x=32, y=32)
                    .bitcast(f32r))
                # coarse restricts (replicated)
                rl = {32: r2f}
                src = r2f
                for s in (32, 16, 8, 4, 2):
                    so = s // 2
                    dstr = TP.tile([so, so, so], f32r, tag=f"r{so}")
                    ps = PSC.tile([so, so, so], f32, tag=f"cr{so}")
                    for i in range(4):
                        dz, dy = i // 2, i % 2
                        nc.tensor.matmul(ps[:], rsc[s][:, i, :],
                                         src[:, dz::2, dy::2][:, 0:so, 0:so],
                                         start=(i == 0), stop=(i == 3))
                    nc.scalar.activation(dstr[:], ps[:], AF.Copy)
                    rl[so] = dstr
                    src = dstr
                if it == iteration - 1:
                    nc.scalar.activation(r7_out[:], r32(rl[1][:, 0, :]), AF.Copy)

                # V-cycle up, replicated to 32
                wcur = TP.tile([1, 1, 1], f32r, tag="w1")
                nc.scalar.activation(wcur[:, 0, :], r32(rl[1][:, 0, :]), AF.Copy,
                                     scale=float(1.0 / diag))
                for s in (1, 2, 4, 8, 16):
                    s2 = 2 * s
                    # prol wcur [s,s,s] -> wpad[s2] interior
                    ps = PSC.tile([s2, s2, s2], f32, tag=f"cp{s2}")
                    rh = wcur[:].rearrange("k z y -> k z 1 y 1").broadcast_to(
                        [s, s, 2, s, 2])
                    nc.tensor.matmul(ps[:], prl[s][:], rh, start=True, stop=True)
                    nc.scalar.activation(wpad[s2][:, 1:s2 + 1, 1:s2 + 1], ps[:],
                                         AF.Copy)
                    # update at s2
                    ps2 = PSC.tile([s2, s2, s2], f32, tag=f"cu{s2}")
                    k = 0
                    nops = 11
                    for dz in range(3):
                        for dy in range(3):
                            nc.tensor.matmul(ps2[:], mco[s2][:, dz * 3 + dy, :],
                                             wpad[s2][:, dz:dz + s2, dy:dy + s2],
                                             start=(k == 0), stop=False)
                            k += 1
                    nc.tensor.matmul(ps2[:], mco[s2][:, 9, :],
                                     wpad[s2][:, 1:s2 + 1, 1:s2 + 1],
                                     start=False, stop=False)
                    nc.tensor.matmul(ps2[:], mco[s2][:, 10, :], rl[s2][:],
                                     start=False, stop=True)
                    wn = TP.tile([s2, s2, s2], f32r, tag=f"wu{s2}")
                    nc.scalar.activation(wn[:], ps2[:], AF.Copy)
                    wcur = wn

                # slice w32 -> per-core 6 planes
                nc.sync.dma_start(
                    w32_d[:].rearrange("(z x) y -> x z y", z=32, x=32),
                    r32(wcur[:]))
                nc.vector.memset(w32sl[:], 0.0)
                for j in range(6):
                    gt = TP.tile([32, 32], f32, tag="w32g")
                    nc.gpsimd.indirect_dma_start(
                        out=gt[:], out_offset=None, in_=w32_d[:],
                        in_offset=bass.IndirectOffsetOnAxis(ap=ixw[:, j:j + 1], axis=0),
                        bounds_check=32 * 32 - 1, oob_is_err=False)
                    nc.vector.tensor_copy(w32sl[:, j, :], gt[:])
                # prol32 -> w64pad (12 planes), 2 chunks
                for h in range(2):
                    ps = PSC.tile([64, 6, 64], f32, tag="c64b")
                    rh = w32sl[:, 3 * h:3 * h + 3, :].rearrange(
                        "k z y -> k z 1 y 1").broadcast_to([32, 3, 2, 32, 2])
                    nc.tensor.matmul(ps[:], prl[32][:], rh, start=True, stop=True)
                    nc.scalar.activation(w64pad[:, 6 * h:6 * h + 6, 1:65], ps[:],
                                         AF.Copy)
                # A64 update on my 8 planes (local slots 2..9)
                ps2 = PSC.tile([64, 8, 64], f32, tag="c64")
                k = 0
                for dz in range(3):
                    for dy in range(3):
                        nc.tensor.matmul(ps2[:], mco[64][:, dz * 3 + dy, :],
                                         w64pad[:, 1 + dz:1 + dz + 8, dy:dy + 64],
                                         start=(k == 0), stop=False)
                        k += 1
                nc.tensor.matmul(ps2[:], mco[64][:, 9, :],
                                 w64pad[:, 2:10, 1:65], start=False, stop=False)
                nc.tensor.matmul(ps2[:], mco[64][:, 10, :], r1[:],
                                 start=False, stop=True)
                w64u = TP.tile([64, 8, 64], f32r, tag="w64u")
                nc.scalar.activation(w64u[:], ps2[:], AF.Copy)

                # p update (+ wmg eviction on last iter): 4 chunks
                last = it == iteration - 1
                for a in range(0, ZL, 4):
                    og = G + a
                    rh64 = w64u[:, a // 2:a // 2 + 2, :].rearrange(
                        "k z y -> k z 1 y 1").broadcast_to([64, 2, 2, 64, 2])
                    if last:
                        psw = PS.tile([D, 4, D], f32, tag="cv")
                        nc.tensor.matmul(psw[:], p64p[:], rh64, start=True, stop=True)
                        nc.scalar.activation(wmg[:, a:a + 4, :], psw[:], AF.Copy)
                    psp = PS.tile([D, 4, D], f32, tag="cv")
                    nc.tensor.matmul(psp[:], ID('I1'),
                                     p[:, og:og + 4, 1:129], start=True, stop=False)
                    nc.tensor.matmul(psp[:], ID('Imdiag'),
                                     App[:, a:a + 4, :], start=False, stop=False)
                    nc.tensor.matmul(psp[:], ID('Ipdiag'),
                                     b[:, a:a + 4, :], start=False, stop=False)
                    nc.tensor.matmul(psp[:], p64n[:], rh64, start=False, stop=True)
                    nc.scalar.activation(p[:, og:og + 4, 1:129], psp[:], AF.Copy)

                # p halo exchange
                nc.sync.dma_start(
                    ph_in[:].rearrange("(p x y) -> p x y", p=2, x=D, y=D)[0],
                    r32(p[:, G:G + 1, 1:129]).rearrange("x o y -> (x o) y"))
                nc.sync.dma_start(
                    ph_in[:].rearrange("(p x y) -> p x y", p=2, x=D, y=D)[1],
                    r32(p[:, G + ZL - 1:G + ZL, 1:129]).rearrange("x o y -> (x o) y"))
                nc.gpsimd.collective_compute(
                    "AllGather", ALU.bypass, replica_groups=[list(range(NC))],
                    ins=[ph_in[:].opt()], outs=[ph_out[:].opt()])
                phv = ph_out[:].rearrange("c (p x y) -> (c p x) y", p=2, x=D, y=D)
                for (col, slot) in ((0, G - 1), (1, G + ZL)):
                    gt = TP.tile([D, D], f32, tag="phg")
                    nc.gpsimd.indirect_dma_start(
                        out=gt[:], out_offset=None, in_=phv,
                        in_offset=bass.IndirectOffsetOnAxis(ap=ixp[:, col:col + 1],
                                                            axis=0))
                    nc.vector.tensor_copy(p[:, slot, 1:129], gt[:])
                ycols(p, 0, G - 1, G + ZL + 1, 1.0)

            # ---------------- final correction ----------------
            mfx = {'u': mset('xp'), 'v': mset('yp'), 'w': mset('zp')}
            for fi, comp in enumerate('uvw'):
                u3 = TP.tile([D, ZL, D], f32, tag="u3")
                mt = mfx[comp]
                for ci, a in enumerate(range(0, ZL, 4)):
                    og = G + a
                    ps = PS.tile([D, 4, D], f32, tag="cv")
                    k = 0
                    for dz in range(3):
                        for dy in range(3):
                            nc.tensor.matmul(ps[:], mt[:, dz * 3 + dy, :],
                                             p[:, og - 1 + dz:og - 1 + dz + 4,
                                               dy:dy + D],
                                             start=(k == 0), stop=False)
                            k += 1
                    qq = U2[comp]
                    nc.tensor.matmul(ps[:], ID('Iminvdt'),
                                     qq[:, og - qq.z0:og - qq.z0 + 4, 1:129],
                                     start=False, stop=True)
                    # u3 = (ps * -dt) * rd
                    nc.vector.scalar_tensor_tensor(
                        u3[:, a:a + 4, :], ps[:], float(-dt),
                        rd[:, og:og + 4, :], op0=ALU.mult, op1=ALU.mult)
                dma(out_f[fi], u3[:])

            dma(out_f[3], r32(p[:, G:G + ZL, 1:129]))
            dma(out_f[4], wmg[:])
            nc.sync.dma_start(out_r[:], r7_out[:])

    nc.compile()
    return nc


# ------------------------------------------------------------------ entry
_CACHE = {}


def kernel(**inputs):
    in_maps, consts = _host_prep(inputs)
    key = (consts['dt'], consts['S'], consts['diag'], consts['iteration'],
           consts['nlevel'],
           hash(np.asarray(inputs['wA'], np.float32).tobytes()),
           hash(np.asarray(inputs['w_res'], np.float32).tobytes()))
    if key not in _CACHE:
        _CACHE[key] = _build(consts)
    nc = _CACHE[key]
    res = bass_utils.run_bass_kernel_spmd(nc, in_maps, core_ids=list(range(NC)))

    def unshard(i):
        return np.concatenate(
            [np.transpose(res.results[c]["out_fields"][i], (1, 2, 0))
             for c in range(NC)], axis=0)[None, None]

    u, v, w, pfin, wmg = (unshard(i) for i in range(5))
    r = res.results[0]["out_r"].reshape(1, 1, 1, 1, 1).astype(np.float32)
    return (u.astype(np.float32), v.astype(np.float32), w.astype(np.float32),
            pfin.astype(np.float32), wmg.astype(np.float32), r)


# revision 20
# speedup vs baseline: 1.0406x; 1.0406x over previous
"""Trainium2 8-core Bass kernel for nn_AI4Urban (CFD step + multigrid).

Self-contained: builds per-call (weights/dt baked as compile-time consts),
shards the 128^3 grid along z across 8 NeuronCores with 3-deep ghost input
planes, runs all 3x3x3 stencils as banded f32r matmuls on the PE
(x in partitions, (z,y) in the free dim), does the multigrid coarse levels
replicated below 64^3 with one AllGather at the 32^3 level plus one
indirect-DMA z-slice per iteration, and exchanges a 1-plane p halo per MG
iteration via AllGather + per-core index gather.
"""
import sys
sys.path.insert(0, '/opt/trn_rl_repo')
import numpy as np

from concourse import bacc, bass, tile, bass_utils, mybir

NC = 8
D = 128
ZL = D // NC        # 16 local planes
G = 3               # ghost depth of input tiles
ZX = ZL + 2 * G     # 22-slot global frame
YP = 130

f32 = mybir.dt.float32
f32r = mybir.dt.float32r
i32 = mybir.dt.int32
AF = mybir.ActivationFunctionType
ALU = mybir.AluOpType

_M128 = {}
_n = 0
for _nm in ('xp', 'yp', 'zp', 'dp_c2', 'dp_c2b', 'Ap',
            'xm', 'ym', 'zm', 'dm_c2', 'dm_c2b'):
    _M128[_nm] = _n
    _n += 9
for _nm in ('I1', 'Ic1', 'Ic1b', 'Imdiag', 'Ipdiag', 'Iminvdt'):
    _M128[_nm] = _n
    _n += 1
N_M128 = _n
MINUS_STENCILS = ('x', 'y', 'z', 'd')


# ------------------------------------------------------------------ host math
def _band(w, f, size=D, fold=True):
    B = (w[0] * np.eye(size, k=1) + w[1] * np.eye(size) + w[2] * np.eye(size, k=-1))
    if fold:
        B[0, 0] += f * w[0]
        B[size - 1, size - 1] += f * w[2]
    return B


def _band_set(w3, f, scale=1.0, size=D, fold=True):
    out = np.zeros((9, size, size), np.float32)
    for dz in range(3):
        for dy in range(3):
            out[dz * 3 + dy] = scale * _band(w3[dz, dy], f, size, fold)
    return out


def _res_set(w_res, s_in):
    so = s_in // 2
    out = np.zeros((4, s_in, so), np.float32)
    for dz in range(2):
        for dy in range(2):
            for m in range(so):
                for dx in range(2):
                    out[dz * 2 + dy, 2 * m + dx, m] = w_res[dz, dy, dx]
    return out


def _prol_mat(s):
    P = np.zeros((s, 2 * s), np.float32)
    for k in range(s):
        P[k, 2 * k] = 1.0
        P[k, 2 * k + 1] = 1.0
    return P


def _host_prep(inputs):
    gv = lambda k: np.asarray(inputs[k], np.float32).reshape(D, D, D)
    vu, vv_, vw, vp = gv('values_u'), gv('values_v'), gv('values_w'), gv('values_p')
    sg = gv('sigma')
    w_x = np.asarray(inputs['w_xadv'], np.float64).reshape(3, 3, 3)
    w_y = np.asarray(inputs['w_yadv'], np.float64).reshape(3, 3, 3)
    w_z = np.asarray(inputs['w_zadv'], np.float64).reshape(3, 3, 3)
    w_d = np.asarray(inputs['w_diff'], np.float64).reshape(3, 3, 3)
    wA = np.asarray(inputs['wA'], np.float64).reshape(3, 3, 3)
    w_res = np.asarray(inputs['w_res'], np.float64).reshape(2, 2, 2)
    dt = float(np.asarray(inputs['dt']).reshape(-1)[0])
    iteration = int(inputs['iteration'])
    nlevel = int(inputs['nlevel'])

    S = float(w_d.sum())
    diag = float(wA[1, 1, 1])
    consts = dict(dt=dt, S=S, diag=diag,
                  c1=1.0 - 0.00025 * dt * S, c2=0.0005 * dt,
                  c2b=0.001 * dt, c1b=-0.0005 * dt * S,
                  iteration=iteration, nlevel=nlevel)

    m128 = np.zeros((N_M128, D, D), np.float32)
    m128[_M128['xp']:_M128['xp'] + 9] = _band_set(w_x, 1.0)
    m128[_M128['yp']:_M128['yp'] + 9] = _band_set(w_y, 1.0)
    m128[_M128['zp']:_M128['zp'] + 9] = _band_set(w_z, 1.0)
    m128[_M128['dp_c2']:_M128['dp_c2'] + 9] = _band_set(w_d, 1.0, consts['c2'])
    m128[_M128['dp_c2b']:_M128['dp_c2b'] + 9] = _band_set(w_d, 1.0, consts['c2b'])
    m128[_M128['Ap']:_M128['Ap'] + 9] = _band_set(wA, 1.0)
    m128[_M128['xm']:_M128['xm'] + 9] = _band_set(w_x, -1.0)
    m128[_M128['ym']:_M128['ym'] + 9] = _band_set(w_y, -1.0)
    m128[_M128['zm']:_M128['zm'] + 9] = _band_set(w_z, -1.0)
    m128[_M128['dm_c2']:_M128['dm_c2'] + 9] = _band_set(w_d, -1.0, consts['c2'])
    m128[_M128['dm_c2b']:_M128['dm_c2b'] + 9] = _band_set(w_d, -1.0, consts['c2b'])
    I = np.eye(D, dtype=np.float32)
    m128[_M128['I1']] = I
    m128[_M128['Ic1']] = consts['c1'] * I
    m128[_M128['Ic1b']] = consts['c1b'] * I
    m128[_M128['Imdiag']] = (-1.0 / diag) * I
    m128[_M128['Ipdiag']] = (1.0 / diag) * I
    m128[_M128['Iminvdt']] = (-1.0 / dt) * I

    base = dict(m128=m128,
                res0=_res_set(w_res, 128), res1=_res_set(w_res, 64),
                prol64p=_prol_mat(64).astype(np.float32),
                prol64n=(-_prol_mat(64)).astype(np.float32))
    for s in (32, 16, 8, 4, 2):
        base[f'resc{s}'] = _res_set(w_res, s)
    for s in (64, 32, 16, 8, 4, 2):
        m = np.zeros((11, s, s), np.float32)
        m[:9] = _band_set(wA, 0.0, -1.0 / diag, s, fold=False)
        m[9] = np.eye(s, dtype=np.float32)
        m[10] = np.eye(s, dtype=np.float32) / diag
        base[f'mco{s}'] = m
    for s in (1, 2, 4, 8, 16, 32):
        base[f'prol{s}'] = _prol_mat(s).astype(np.float32)

    ws = {'x': w_x, 'y': w_y, 'z': w_z, 'd': w_d}
    L12 = np.zeros((4, 2, 12, D), np.float32)
    for si, sn in enumerate(MINUS_STENCILS):
        w3 = ws[sn]
        for yc in range(2):
            dyt = 0 if yc == 0 else 2
            for dz in range(3):
                for xe in range(2):
                    m = 0 if xe == 0 else D - 1
                    xet = 0 if xe == 0 else 2
                    L12[si, yc, dz * 4 + yc * 2 + xe, m] = 2.0 * w3[dz, dyt, xet]
    base['L12'] = L12


    def shard(gf):
        out = []
        for c in range(NC):
            zmin = c * ZL - G
            idx = np.clip(np.arange(zmin, zmin + ZX), 0, D - 1)
            out.append(np.ascontiguousarray(
                np.transpose(gf[idx], (2, 0, 1)).astype(np.float32)))
        return out

    sh_u, sh_v, sh_w, sh_p, sh_s = (shard(a) for a in (vu, vv_, vw, vp, sg))

    in_maps = []
    for c in range(NC):
        is_bot, is_top = c == 0, c == NC - 1
        vmv = np.zeros(10, np.float32)
        vmv[0] = 0.0 if is_bot else 1.0
        vmv[1] = -1.0 if is_bot else 0.0
        vmv[2] = 1.0 if is_bot else 0.0
        vmv[3] = 0.0 if is_top else 1.0
        vmv[4] = -1.0 if is_top else 0.0
        vmv[5] = 1.0 if is_top else 0.0
        vmv[6] = -1.0 if is_bot else 1.0
        vmv[7] = -1.0 if is_top else 1.0
        vmv[8] = 1.0 if is_bot else -1.0
        vmv[9] = 1.0 if is_top else -1.0

        corrb = np.zeros((D, 24), np.float32)
        for si, sn in enumerate(MINUS_STENCILS):
            w3 = ws[sn]
            for side in range(2):
                mask = 1.0 if (is_bot if side == 0 else is_top) else 0.0
                dzt = 0 if side == 0 else 2
                for dy in range(3):
                    col = si * 6 + side * 3 + dy
                    corrb[0, col] = mask * 2.0 * w3[dzt, dy, 0]
                    corrb[D - 1, col] = mask * 2.0 * w3[dzt, dy, 2]

        xs = np.arange(D, dtype=np.int32)
        bot_pl = 2 * (c - 1) + 1 if c > 0 else 0
        top_pl = 2 * (c + 1) + 0 if c < NC - 1 else 2 * c + 1
        idx_ph = np.stack([bot_pl * D + xs, top_pl * D + xs], 1).astype(np.int32)

        idx_w = np.zeros((32, 6), np.int32)
        for j, z in enumerate(range(4 * c - 1, 4 * c + 5)):
            idx_w[:, j] = (z * 32 + np.arange(32)) if 0 <= z < 32 else (32 * 32 + np.arange(32))

        m = dict(base)
        m.update(fld_u=sh_u[c], fld_v=sh_v[c], fld_w=sh_w[c], fld_p=sh_p[c],
                 fld_s=sh_s[c], vm=vmv, idx_ph=idx_ph, idx_w=idx_w)
        in_maps.append(m)
    return in_maps, consts


# ------------------------------------------------------------------ builder
def _build(consts):
    dt = consts['dt']
    diag = consts['diag']
    iteration = consts['iteration']

    nc = bacc.Bacc("TRN2", target_bir_lowering=False, debug=False, num_devices=NC)
    dr = {}

    def din(name, shape, dtp=f32r):
        dr[name] = nc.dram_tensor(name, list(shape), dtp, kind="ExternalInput")

    for nm in ('fld_u', 'fld_v', 'fld_w', 'fld_p'):
        din(nm, (D, ZX, D))
    din('fld_s', (D, ZX, D), f32)
    din('m128', (N_M128, D, D))
    din('res0', (4, 128, 64), f32); din('res1', (4, 64, 32), f32)
    for s in (32, 16, 8, 4, 2):
        din(f'resc{s}', (4, s, s // 2), f32)
    for s in (64, 32, 16, 8, 4, 2):
        din(f'mco{s}', (11, s, s))
    for s in (1, 2, 4, 8, 16, 32):
        din(f'prol{s}', (s, 2 * s), f32)
    din('prol64p', (64, 128), f32); din('prol64n', (64, 128), f32)
    din('L12', (4, 2, 12, D), f32)
    din('vm', (10,), f32)
    din('idx_ph', (D, 2), i32); din('idx_w', (32, 6), i32)

    out_f = nc.dram_tensor("out_fields", [5, D, ZL, D], f32, kind="ExternalOutput")
    out_r = nc.dram_tensor("out_r", [1, 1], f32, kind="ExternalOutput")

    DMAE = [nc.sync, nc.scalar, nc.gpsimd]
    dma_i = [0]

    def dma(dst, src):
        DMAE[dma_i[0] % len(DMAE)].dma_start(dst, src)
        dma_i[0] += 1

    with tile.TileContext(nc) as tc:
        with (
            tc.tile_pool(name="pool", bufs=1) as P_,
            tc.tile_pool(name="mats", bufs=3) as MP,
            tc.tile_pool(name="tmp", bufs=2) as TP,
            tc.tile_pool(name="psum", bufs=4, space="PSUM") as PS,
            tc.tile_pool(name="psc", bufs=2, space="PSUM") as PSC,
            tc.tile_pool(name="dram", bufs=1, space="DRAM") as DP,
        ):
            r32 = lambda ap: ap.bitcast(f32)

            ids = P_.tile([D, 6, D], f32r, tag="ids")
            dma(ids[:], dr['m128'][_M128['I1']:_M128['I1'] + 6]
                .rearrange("i k m -> k i m"))

            def ID(nm):
                return ids[:, _M128[nm] - _M128['I1'], :]

            rs0 = P_.tile([128, 4, 64], f32, tag="rs0")
            dma(rs0[:], dr['res0'][:].rearrange("i k m -> k i m"))
            rs1 = P_.tile([64, 4, 32], f32, tag="rs1")
            dma(rs1[:], dr['res1'][:].rearrange("i k m -> k i m"))
            rsc, mco, prl = {}, {}, {}
            for s in (32, 16, 8, 4, 2):
                rsc[s] = P_.tile([s, 4, s // 2], f32, tag=f"rsc{s}", name=f"rsc{s}")
                dma(rsc[s][:], dr[f'resc{s}'][:].rearrange("i k m -> k i m"))
            for s in (64, 32, 16, 8, 4, 2):
                mco[s] = P_.tile([s, 11, s], f32r, tag=f"mco{s}", name=f"mco{s}")
                dma(mco[s][:], dr[f'mco{s}'][:].rearrange("i k m -> k i m"))
            for s in (1, 2, 4, 8, 16, 32):
                prl[s] = P_.tile([s, 2 * s], f32, tag=f"prl{s}", name=f"prl{s}")
                dma(prl[s][:], dr[f'prol{s}'][:])
            p64p = P_.tile([64, 128], f32, tag="p64p")
            dma(p64p[:], dr['prol64p'][:])
            p64n = P_.tile([64, 128], f32, tag="p64n")
            dma(p64n[:], dr['prol64n'][:])
            L12 = P_.tile([12, 8, D], f32, tag="L12")
            dma(L12[:], dr['L12'][:].rearrange("s c k m -> k (s c) m"))

            vm = P_.tile([D, 10], f32, tag="vm")
            dma(vm[:], dr['vm'][:].partition_broadcast(D))
            ixp = P_.tile([D, 2], i32, tag="ixp")
            dma(ixp[:], dr['idx_ph'][:])
            ixw = P_.tile([32, 6], i32, tag="ixw")
            dma(ixw[:], dr['idx_w'][:])


            class FT:
                def __init__(self, ap, z0):
                    self.ap = ap
                    self.z0 = z0
                    self.shape = ap.shape

                def __getitem__(self, k):
                    return self.ap[k]

            def ftile(tag, z0, zn):
                t = P_.tile([D, zn, YP], f32r, tag=tag, name=tag)
                return FT(t, z0)

            U = {n: ftile('t' + n, 0, ZX) for n in 'uvw'}
            p = ftile('p', 0, ZX)
            rd = P_.tile([D, ZX, D], f32, tag="rd")
            B_ = {n: ftile('b' + n, 1, 20) for n in 'uvw'}
            U2 = {n: ftile('q' + n, 2, 18) for n in 'uvw'}
            xps = {}
            for n in 'uvw':
                t = P_.tile([D, 20, D], mybir.dt.bfloat16, tag='xp' + n,
                            name='xp' + n)
                xps[n] = FT(t, 1)

            for n, src in (('u', 'fld_u'), ('v', 'fld_v'), ('w', 'fld_w')):
                dma(U[n][:, :, 1:129], dr[src][:])
            dma(p[:, :, 1:129], dr['fld_p'][:])
            dma(rd[:], dr['fld_s'][:])
            nc.vector.tensor_scalar(rd[:], rd[:], float(dt), 1.0,
                                    op0=ALU.mult, op1=ALU.add)
            nc.vector.reciprocal(rd[:], rd[:])
            for n in 'uvw':
                nc.vector.tensor_mul(U[n][:, :, 1:129], r32(U[n][:, :, 1:129]), rd[:])
            for (sl, vi) in ((slice(0, G), 6), (slice(ZX - G, ZX), 7)):
                nc.scalar.activation(U['u'][:, sl, 1:129], r32(U['u'][:, sl, 1:129]),
                                     AF.Copy, scale=vm[:, vi:vi + 1])

            def ycols(t, a, b_, f, gslots=()):
                la, lb = a - t.z0, b_ - t.z0
                nc.scalar.activation(t[:, la:lb, 0:1], r32(t[:, la:lb, 1:2]),
                                     AF.Copy, scale=float(f))
                nc.scalar.activation(t[:, la:lb, 129:130], r32(t[:, la:lb, 128:129]),
                                     AF.Copy, scale=float(f))
                for (g, vi) in gslots:
                    lg = g - t.z0
                    nc.scalar.activation(t[:, lg:lg + 1, 0:1],
                                         r32(t[:, lg:lg + 1, 1:2]),
                                         AF.Copy, scale=vm[:, vi:vi + 1])
                    nc.scalar.activation(t[:, lg:lg + 1, 129:130],
                                         r32(t[:, lg:lg + 1, 128:129]),
                                         AF.Copy, scale=vm[:, vi:vi + 1])

            ycols(U['u'], G, ZX - G, -1.0,
                  gslots=[(g, 8) for g in range(G)]
                  + [(g, 9) for g in range(ZX - G, ZX)])
            ycols(U['v'], 0, ZX, 1.0)
            ycols(U['w'], 0, ZX, 1.0)
            ycols(p, 0, ZX, 1.0)

            def cc3(t, o0, zout):
                c = TP.tile([12, zout], f32r, tag="cc3")
                for dz in range(3):
                    for yc in range(2):
                        for xe in range(2):
                            k = dz * 4 + yc * 2 + xe
                            m = 0 if xe == 0 else D - 1
                            ycol = 0 if yc == 0 else 129
                            ls = o0 - 1 + dz - t.z0
                            nc.sync.dma_start(
                                c[k:k + 1, 0:zout],
                                t[m:m + 1, ls:ls + zout, ycol:ycol + 1]
                                .rearrange("p z o -> p (z o)"))
                return c

            def mset(nm):
                t = MP.tile([D, 9, D], f32r, tag="mset")
                b0 = _M128[nm]
                dma(t[:], dr['m128'][b0:b0 + 9].rearrange("i k m -> k i m"))
                return t

            def conv_groups(T, o0, zout, mt, id_terms=(), corr=None):
                for a in range(0, zout, 4):
                    cp = min(4, zout - a)
                    ps = PS.tile([D, 4, D], f32, tag="cv")
                    ops = []
                    for dz in range(3):
                        for dy in range(3):
                            ls = o0 + a - 1 + dz - T.z0
                            ops.append((mt[:, dz * 3 + dy, :],
                                        T[:, ls:ls + cp, dy:dy + D],
                                        ps[:, 0:cp, :]))
                    for (iap, src) in id_terms:
                        lo = o0 + a - src.z0
                        ops.append((iap, src[:, lo:lo + cp, 1:129], ps[:, 0:cp, :]))
                    if corr is not None:
                        si, cct, srcT = corr
                        for yc in range(2):
                            yo = 0 if yc == 0 else D - 1
                            ops.append((L12[:, si * 2 + yc, :],
                                        cct[:, a:a + cp].bitcast(f32),
                                        ps[:, 0:cp, yo:yo + 1]))
                    n = len(ops)
                    for i, (lh, rh, o_ap) in enumerate(ops):
                        nc.tensor.matmul(o_ap, lh, rh,
                                         start=(i == 0), stop=(i == n - 1))
                    yield a, o0 + a, cp, ps

            def full_conv(T, o0, zout, mt, dst_fn, scale=1.0):
                for a, og, cp, ps in conv_groups(T, o0, zout, mt):
                    nc.scalar.activation(dst_fn(a, cp), ps[:, 0:cp, :],
                                         AF.Copy, scale=float(scale))

            # ---------------- stage 1 + 2 ----------------
            def stage_combine(comp, T, o0, zout, PROD, msets, mdf, id_terms,
                              dtc, xpsrc, dst, cc):
                is_minus = comp == 'u'
                mtile = TP.tile([D, zout, D], f32, tag="madv")
                for k, (sn, mul) in enumerate((('x', 'u'), ('y', 'v'), ('z', 'w'))):
                    corr = (MINUS_STENCILS.index(sn), cc, T) if is_minus else None
                    mt_k = mset(msets[sn])
                    for a, og, cp, ps in conv_groups(T, o0, zout, mt_k,
                                                     corr=corr):
                        mu = PROD[mul]
                        msl = r32(mu[:, og - mu.z0: og - mu.z0 + cp, 1:129])
                        if k == 0:
                            nc.vector.tensor_mul(mtile[:, a:a + cp, :], msl,
                                                 ps[:, 0:cp, :])
                        else:
                            tt = TP.tile([D, 4, D], f32, tag="advt", bufs=1)
                            nc.vector.tensor_mul(tt[:, 0:cp, :], msl, ps[:, 0:cp, :])
                            nc.gpsimd.tensor_add(mtile[:, a:a + cp, :],
                                                 mtile[:, a:a + cp, :], tt[:, 0:cp, :])
                corr = (3, cc, T) if is_minus else None
                mt_d = mset(mdf)
                for a, og, cp, ps in conv_groups(T, o0, zout, mt_d,
                                                 id_terms=id_terms, corr=corr):
                    nc.vector.scalar_tensor_tensor(
                        mtile[:, a:a + cp, :], mtile[:, a:a + cp, :], float(-dtc),
                        ps[:, 0:cp, :], op0=ALU.mult, op1=ALU.add)
                nc.gpsimd.tensor_add(mtile[:], mtile[:],
                                     xpsrc[:, o0 - xpsrc.z0:o0 - xpsrc.z0 + zout, :])
                nc.vector.tensor_mul(dst[:, o0 - dst.z0:o0 - dst.z0 + zout, 1:129],
                                     mtile[:], rd[:, o0:o0 + zout, :])

            for sn, n in (('xp', 'u'), ('yp', 'v'), ('zp', 'w')):
                mt = mset(sn)
                xx = xps[n]
                full_conv(p, 1, 20, mt,
                          lambda a, cp, _x=xx: _x[:, a:a + cp, :], scale=-dt)

            cc_u = cc3(U['u'], 1, 20)
            for comp in 'uvw':
                ms = ({'x': 'xm', 'y': 'ym', 'z': 'zm'} if comp == 'u'
                      else {'x': 'xp', 'y': 'yp', 'z': 'zp'})
                stage_combine(comp, U[comp], 1, 20, U, ms,
                              'dm_c2' if comp == 'u' else 'dp_c2',
                              [(ID('Ic1'), U[comp])],
                              0.5 * dt, xps[comp], B_[comp],
                              cc_u if comp == 'u' else None)

            def blend(t, slots_bot, slots_top, minus):
                eb = t[:, G - t.z0:G - t.z0 + 1, 1:129]
                et = t[:, G + ZL - 1 - t.z0:G + ZL - t.z0, 1:129]
                for (slots, edge, mvi, svi) in (
                        (slots_bot, eb, 0, 1 if minus else 2),
                        (slots_top, et, 3, 4 if minus else 5)):
                    for g in slots:
                        sl = t[:, g - t.z0:g - t.z0 + 1, 1:129]
                        nc.scalar.activation(sl, r32(sl), AF.Copy,
                                             scale=vm[:, mvi:mvi + 1])
                        nc.vector.scalar_tensor_tensor(
                            sl, r32(edge), vm[:, svi:svi + 1], r32(sl),
                            op0=ALU.mult, op1=ALU.add)

            blend(B_['u'], (1, 2), (19, 20), True)
            blend(B_['v'], (1, 2), (19, 20), False)
            blend(B_['w'], (1, 2), (19, 20), False)
            ycols(B_['u'], G, ZX - G, -1.0,
                  gslots=[(1, 8), (2, 8), (19, 9), (20, 9)])
            ycols(B_['v'], 1, 21, 1.0)
            ycols(B_['w'], 1, 21, 1.0)

            cc_b = cc3(B_['u'], 2, 18)
            for comp in 'uvw':
                ms = ({'x': 'xm', 'y': 'ym', 'z': 'zm'} if comp == 'u'
                      else {'x': 'xp', 'y': 'yp', 'z': 'zp'})
                stage_combine(comp, B_[comp], 2, 18, B_, ms,
                              'dm_c2b' if comp == 'u' else 'dp_c2b',
                              [(ID('Ic1b'), B_[comp]), (ID('I1'), U[comp])],
                              dt, xps[comp], U2[comp],
                              cc_b if comp == 'u' else None)

            blend(U2['u'], (2,), (19,), True)
            blend(U2['v'], (2,), (19,), False)
            blend(U2['w'], (2,), (19,), False)
            ycols(U2['u'], G, ZX - G, -1.0, gslots=[(2, 8), (19, 9)])
            ycols(U2['v'], 2, 20, 1.0)
            ycols(U2['w'], 2, 20, 1.0)

            # ---------------- stage 3: b ----------------
            b = FT(P_.tile([D, ZL, D], f32r, tag="bu", name="stb"), G)
            mt_x3, mt_y3, mt_z3 = mset('xm'), mset('yp'), mset('zp')
            cc_q = cc3(U2['u'], G, ZL)
            for a in range(0, ZL, 4):
                og = G + a
                ps = PS.tile([D, 4, D], f32, tag="cv")
                ops = []
                for (mt, T) in ((mt_x3, U2['u']), (mt_y3, U2['v']), (mt_z3, U2['w'])):
                    for dz in range(3):
                        for dy in range(3):
                            ls = og - 1 + dz - T.z0
                            ops.append((mt[:, dz * 3 + dy, :],
                                        T[:, ls:ls + 4, dy:dy + D], ps[:]))
                for yc in range(2):
                    yo = 0 if yc == 0 else D - 1
                    ops.append((L12[:, 0 * 2 + yc, :],
                                cc_q[:, a:a + 4].bitcast(f32),
                                ps[:, :, yo:yo + 1]))
                n = len(ops)
                for i, (lh, rh, o_ap) in enumerate(ops):
                    nc.tensor.matmul(o_ap, lh, rh, start=(i == 0), stop=(i == n - 1))
                nc.scalar.activation(b[:, a:a + 4, :], ps[:], AF.Copy,
                                     scale=float(-1.0 / dt))

            # ---------------- multigrid ----------------
            App = FT(P_.tile([D, ZL, D], f32r, tag="bv", name="app"), G)
            wmg = P_.tile([D, ZL, D], f32, tag="bw")


            wpad = {}
            for s in (4, 8, 16, 32):
                wpad[s] = P_.tile([s, s + 2, s + 2], f32r, tag=f"wp{s}", name=f"wp{s}")
                nc.vector.memset(wpad[s][:].bitcast(f32), 0.0)
                nc.vector.tensor_copy(wpad[s][:], wpad[s][:].bitcast(f32))
            wpad[2] = P_.tile([2, 4, 4], f32r, tag="wp2", name="wp2")
            nc.vector.memset(wpad[2][:].bitcast(f32), 0.0)
            nc.vector.tensor_copy(wpad[2][:], wpad[2][:].bitcast(f32))
            w64pad = P_.tile([64, 12, 66], f32r, tag="w64p")
            nc.vector.memset(w64pad[:].bitcast(f32), 0.0)
            nc.vector.tensor_copy(w64pad[:], w64pad[:].bitcast(f32))
            w32sl = P_.tile([32, 6, 32], f32r, tag="w32sl")
            zrow32 = P_.tile([32, 32], f32, tag="zrow32")
            nc.vector.memset(zrow32[:], 0.0)
            r7_out = P_.tile([1, 1], f32, tag="r7o")

            def prol_mm(lh, src, zsl, parts, zn, width, ps, first, last):
                """prol: out[2z+a, 2y+b] = src[z, y]; 2 matmuls (a=0,1)."""
                pv = ps[:].rearrange("m (z a) y -> m a z y", a=2)
                rh = (src[:, zsl, :].bitcast(f32).unsqueeze(3)
                      .broadcast_to([parts, zn, width, 2]))
                for a in range(2):
                    nc.tensor.matmul(pv[:, a], lh, rh,
                                     start=(first and a == 0),
                                     stop=(last and a == 1))

            for it in range(iteration):
                ph_in = DP.tile([2 * D * D], f32, tag=f"phin{it}", name=f"phin{it}")
                ph_out = DP.tile([NC, 2 * D * D], f32, tag=f"phout{it}",
                                 name=f"phout{it}", addr_space="Shared")
                r2_in = DP.tile([4 * 32 * 32], f32, tag=f"r2in{it}",
                                name=f"r2in{it}")
                r2_out = DP.tile([NC, 4 * 32 * 32], f32, tag=f"r2out{it}",
                                 name=f"r2out{it}", addr_space="Shared")
                w32_d = DP.tile([33 * 32, 32], f32, tag=f"w32d{it}",
                                name=f"w32d{it}")
                apm = mset('Ap')
                full_conv(p, G, ZL, apm,
                          lambda a, cp: App[:, a:a + cp, :])
                rzero = P_.tile([D, ZL, D], f32r, tag="tv")
                nc.vector.tensor_sub(rzero[:], r32(App[:]), r32(b[:]))
                r1 = TP.tile([64, 8, 64], f32r, tag="r1", bufs=1)
                ps = PSC.tile([64, 8, 64], f32, tag="co")
                rzv = r32(rzero[:]).rearrange("k (z a) (y c) -> k a c z y", a=2, c=2)
                for i in range(4):
                    nc.tensor.matmul(ps[:], rs0[:, i, :],
                                     rzv[:, i // 2, i % 2],
                                     start=(i == 0), stop=(i == 3))
                nc.scalar.activation(r1[:], ps[:], AF.Copy)
                r2 = TP.tile([32, 4, 32], f32r, tag="r2", bufs=1)
                ps = PSC.tile([32, 4, 32], f32, tag="co")
                r1v = r32(r1[:]).rearrange("k (z a) (y c) -> k a c z y", a=2, c=2)
                for i in range(4):
                    nc.tensor.matmul(ps[:], rs1[:, i, :],
                                     r1v[:, i // 2, i % 2],
                                     start=(i == 0), stop=(i == 3))
                nc.scalar.activation(r2[:], ps[:], AF.Copy)

                nc.sync.dma_start(
                    r2_in[:].rearrange("(z x y) -> x z y", z=4, x=32, y=32),
                    r32(r2[:]))
                nc.gpsimd.collective_compute(
                    "AllGather", ALU.bypass, replica_groups=[list(range(NC))],
                    ins=[r2_in[:].opt()], outs=[r2_out[:].opt()])
                r2f = TP.tile([32, 32, 32], f32r, tag="r2f", bufs=1)
                nc.sync.dma_start(
                    r2f[:],
                    r2_out[:].rearrange("c (z x y) -> x (c z) y", z=4, x=32, y=32)
                    .bitcast(f32r))

                rl = {32: r2f}
                src = r2f
                for s in (32, 16, 8, 4, 2):
                    so = s // 2
                    dstr = TP.tile([so, so, so], f32r, tag=f"rv{so}", name=f"rv{so}")
                    ps = PSC.tile([so, so, so], f32, tag="co")
                    sv = r32(src[:]).rearrange("k (z a) (y c) -> k a c z y",
                                               a=2, c=2)
                    for i in range(4):
                        nc.tensor.matmul(ps[:], rsc[s][:, i, :],
                                         sv[:, i // 2, i % 2],
                                         start=(i == 0), stop=(i == 3))
                    nc.scalar.activation(dstr[:], ps[:], AF.Copy)
                    rl[so] = dstr
                    src = dstr
                if it == iteration - 1:
                    nc.scalar.activation(r7_out[:].unsqueeze(1), r32(rl[1][:]),
                                         AF.Copy)

                wcur = TP.tile([1, 1, 1], f32r, tag="w1")
                nc.scalar.activation(wcur[:], r32(rl[1][:]), AF.Copy,
                                     scale=float(1.0 / diag))
                for s in (1, 2, 4, 8, 16):
                    s2 = 2 * s
                    nhalf = 2 if s2 == 32 else 1
                    for h in range(nhalf):
                        zh = s2 // nhalf
                        ps = PSC.tile([s2, zh, s2], f32, tag="co")
                        prol_mm(prl[s][:], wcur,
                                slice(h * zh // 2, (h + 1) * zh // 2),
                                s, zh // 2, s, ps, True, True)
                        nc.scalar.activation(
                            wpad[s2][:, 1 + h * zh:1 + (h + 1) * zh, 1:s2 + 1],
                            ps[:], AF.Copy)
                    wn = TP.tile([32, 32, 32], f32r, tag="wu", name=f"wu{s2}", bufs=1)[0:s2, 0:s2, 0:s2]
                    for h in range(nhalf):
                        zh = s2 // nhalf
                        ps2 = PSC.tile([s2, zh, s2], f32, tag="co")
                        k = 0
                        for dz in range(3):
                            for dy in range(3):
                                nc.tensor.matmul(
                                    ps2[:], mco[s2][:, dz * 3 + dy, :],
                                    wpad[s2][:, h * zh + dz:h * zh + dz + zh,
                                             dy:dy + s2],
                                    start=(k == 0), stop=False)
                                k += 1
                        nc.tensor.matmul(ps2[:], mco[s2][:, 9, :],
                                         wpad[s2][:, 1 + h * zh:1 + (h + 1) * zh,
                                                  1:s2 + 1],
                                         start=False, stop=False)
                        nc.tensor.matmul(ps2[:], mco[s2][:, 10, :],
                                         rl[s2][:, h * zh:(h + 1) * zh, :],
                                         start=False, stop=True)
                        nc.scalar.activation(wn[:, h * zh:(h + 1) * zh, :], ps2[:],
                                             AF.Copy)
                    wcur = wn

                nc.sync.dma_start(
                    w32_d[0:1024].rearrange("(z x) y -> x z y", z=32, x=32),
                    r32(wcur[:]))
                nc.sync.dma_start(w32_d[1024:1056], zrow32[:])
                nc.vector.memset(w32sl[:].bitcast(f32), 0.0)
                nc.vector.tensor_copy(w32sl[:], w32sl[:].bitcast(f32))
                for j in range(6):
                    gt = TP.tile([D, D], f32, tag="gat", name="gt32")[0:32, 0:32]
                    nc.gpsimd.indirect_dma_start(
                        out=gt[:], out_offset=None, in_=w32_d[:],
                        in_offset=bass.IndirectOffsetOnAxis(ap=ixw[:, j:j + 1],
                                                            axis=0))
                    nc.vector.tensor_copy(w32sl[:, j:j + 1, :], gt[:].unsqueeze(1))
                for h in range(2):
                    ps = PSC.tile([64, 6, 64], f32, tag="co")
                    prol_mm(prl[32][:], w32sl, slice(3 * h, 3 * h + 3),
                            32, 3, 32, ps, True, True)
                    nc.scalar.activation(w64pad[:, 6 * h:6 * h + 6, 1:65], ps[:],
                                         AF.Copy)
                ps2 = PSC.tile([64, 8, 64], f32, tag="co")
                k = 0
                for dz in range(3):
                    for dy in range(3):
                        nc.tensor.matmul(ps2[:], mco[64][:, dz * 3 + dy, :],
                                         w64pad[:, 1 + dz:1 + dz + 8, dy:dy + 64],
                                         start=(k == 0), stop=False)
                        k += 1
                nc.tensor.matmul(ps2[:], mco[64][:, 9, :], w64pad[:, 2:10, 1:65],
                                 start=False, stop=False)
                nc.tensor.matmul(ps2[:], mco[64][:, 10, :], r1[:],
                                 start=False, stop=True)
                w64u = TP.tile([64, 8, 64], f32r, tag="w64u", bufs=1)
                nc.scalar.activation(w64u[:], ps2[:], AF.Copy)

                last = it == iteration - 1
                for a in range(0, ZL, 4):
                    og = G + a
                    if last:
                        psw = PS.tile([D, 4, D], f32, tag="cv")
                        prol_mm(p64p[:], w64u, slice(a // 2, a // 2 + 2),
                                64, 2, 64, psw, True, True)
                        nc.scalar.activation(wmg[:, a:a + 4, :], psw[:], AF.Copy)
                    psp = PS.tile([D, 4, D], f32, tag="cv")
                    nc.tensor.matmul(psp[:], ID('I1'), p[:, og:og + 4, 1:129],
                                     start=True, stop=False)
                    nc.tensor.matmul(psp[:], ID('Imdiag'), App[:, a:a + 4, :],
                                     start=False, stop=False)
                    nc.tensor.matmul(psp[:], ID('Ipdiag'), b[:, a:a + 4, :],
                                     start=False, stop=False)
                    prol_mm(p64n[:], w64u, slice(a // 2, a // 2 + 2),
                            64, 2, 64, psp, False, True)
                    nc.scalar.activation(p[:, og:og + 4, 1:129], psp[:], AF.Copy)

                nc.sync.dma_start(
                    ph_in[0:D * D].rearrange("(x y) -> x y", x=D),
                    r32(p[:, G:G + 1, 1:129]).rearrange("x o y -> x (o y)"))
                nc.sync.dma_start(
                    ph_in[D * D:2 * D * D].rearrange("(x y) -> x y", x=D),
                    r32(p[:, G + ZL - 1:G + ZL, 1:129])
                    .rearrange("x o y -> x (o y)"))
                nc.gpsimd.collective_compute(
                    "AllGather", ALU.bypass, replica_groups=[list(range(NC))],
                    ins=[ph_in[:].opt()], outs=[ph_out[:].opt()])
                phv = ph_out[:].rearrange("c (p x y) -> (c p x) y", p=2, x=D, y=D)
                for (col, slot) in ((0, G - 1), (1, G + ZL)):
                    gt = TP.tile([D, D], f32, tag="gat")
                    nc.gpsimd.indirect_dma_start(
                        out=gt[:], out_offset=None, in_=phv,
                        in_offset=bass.IndirectOffsetOnAxis(
                            ap=ixp[:, col:col + 1], axis=0))
                    nc.vector.tensor_copy(p[:, slot:slot + 1, 1:129],
                                          gt[:].unsqueeze(1))
                ycols(p, G - 1, G + ZL + 1, 1.0)

            # ---------------- final correction ----------------
            for fi, comp in enumerate('uvw'):
                u3 = P_.tile([D, ZL, D], f32, tag="tu", name="u3")
                mt = mset({'u': 'xp', 'v': 'yp', 'w': 'zp'}[comp])
                for a in range(0, ZL, 4):
                    og = G + a
                    ps = PS.tile([D, 4, D], f32, tag="cv")
                    k = 0
                    for dz in range(3):
                        for dy in range(3):
                            nc.tensor.matmul(
                                ps[:], mt[:, dz * 3 + dy, :],
                                p[:, og - 1 + dz:og - 1 + dz + 4, dy:dy + D],
                                start=(k == 0), stop=False)
                            k += 1
                    qq = U2[comp]
                    nc.tensor.matmul(ps[:], ID('Iminvdt'),
                                     qq[:, og - qq.z0:og - qq.z0 + 4, 1:129],
                                     start=False, stop=True)
                    nc.vector.scalar_tensor_tensor(
                        u3[:, a:a + 4, :], ps[:], float(-dt),
                        rd[:, og:og + 4, :], op0=ALU.mult, op1=ALU.mult)
                dma(out_f[fi], u3[:])

            dma(out_f[3], r32(p[:, G:G + ZL, 1:129]))
            dma(out_f[4], wmg[:])
            nc.sync.dma_start(out_r[:], r7_out[:])

    nc.compile()
    return nc


# ------------------------------------------------------------------ entry
_CACHE = {}


def kernel(**inputs):
    in_maps, consts = _host_prep(inputs)
    key = (consts['dt'], consts['S'], consts['diag'], consts['iteration'],
           consts['nlevel'],
           np.asarray(inputs['wA'], np.float32).tobytes(),
           np.asarray(inputs['w_res'], np.float32).tobytes())
    if key not in _CACHE:
        _CACHE[key] = _build(consts)
    res = bass_utils.run_bass_kernel_spmd(_CACHE[key], in_maps,
                                          core_ids=list(range(NC)))

    def unshard(i):
        return np.concatenate(
            [np.transpose(res.results[c]["out_fields"][i], (1, 2, 0))
             for c in range(NC)], axis=0)[None, None]

    u, v, w, pfin, wmg = (unshard(i) for i in range(5))
    r = res.results[0]["out_r"].reshape(1, 1, 1, 1, 1).astype(np.float32)
    return (u.astype(np.float32), v.astype(np.float32), w.astype(np.float32),
            pfin.astype(np.float32), wmg.astype(np.float32), r)
